# revision 1
# baseline (speedup 1.0000x reference)
"""Transformer block (pre-norm attention + MLP) on 8 TRN2 NeuronCores.

Sharding: 8 cores = 4 batch elements x 2 sequence halves (data parallel, no
collectives). Each core computes its 1024 "own" query tokens end-to-end and
redundantly builds K/V for the full 2048-token batch element. The k-token
order is permuted per core (own tokens first) so the SPMD program is
identical on every core — softmax over k is permutation invariant.

All matmuls run with bf16 operands (fp32 PSUM accumulation); the residual
path stays fp32. LayerNorm affine params are folded into the adjacent matmul
weights host-side. Softmax skips max-subtraction (|scores| <= ~10 here) and
gets its denominators for free from an appended ones-column on V.
"""

from collections import deque
from contextlib import ExitStack

import numpy as np

try:
    import jax
    jax.config.update("jax_compilation_cache_dir", "/tmp/jax_bass_cache")
    jax.config.update("jax_persistent_cache_min_compile_time_secs", 0.0)
    jax.config.update("jax_persistent_cache_min_entry_size_bytes", -1)
except Exception:
    import jax

import concourse.bacc as bacc
import concourse.bass as bass
import concourse.mybir as mybir
import concourse.tile as tile
from concourse.masks import make_identity

FP32 = mybir.dt.float32
BF16 = mybir.dt.bfloat16
AF = mybir.ActivationFunctionType
ALU = mybir.AluOpType

D = 1024          # model dim
DT = 8            # d tiles of 128
H = 16            # heads
HD = 64           # head dim
HID = 4096        # mlp hidden
T_ALL = 2048      # tokens per core incl. K/V-only tokens
T_OWN = 1024      # query/output tokens per core
EPS = 1e-6
N_CORES = 8


def _ln_transpose(nc, statp, znp, trp, src_getter, n_tiles, zt_out, eps_sb, ident,
                  copies_on_act=False, apply_on_gpsimd=False):
    """LayerNorm (w/b folded into the following matmul weights host-side)
    + PE transpose into zt_out [128, DT, n_tiles*128] bf16."""
    for tt in range(n_tiles):
        xt = src_getter(tt)  # [128, D] fp32 sbuf tile
        stats = statp.tile([128, 2, 6], FP32, tag="stats")
        nc.vector.bn_stats(out=stats[:, 0, :], in_=xt[:, 0:512])
        nc.vector.bn_stats(out=stats[:, 1, :], in_=xt[:, 512:1024])
        mv = statp.tile([128, 2], FP32, tag="mv")
        nc.vector.bn_aggr(out=mv, in_=stats)
        sd = statp.tile([128, 1], FP32, tag="sd")
        nc.scalar.activation(out=sd, in_=mv[:, 1:2], func=AF.Sqrt, bias=eps_sb)
        rinv = statp.tile([128, 1], FP32, tag="rinv")
        nc.vector.reciprocal(out=rinv, in_=sd)
        zn = znp.tile([128, D], BF16, tag="zn")
        eng = nc.gpsimd if apply_on_gpsimd else nc.vector
        eng.tensor_scalar(
            out=zn, in0=xt, scalar1=mv[:, 0:1], scalar2=rinv,
            op0=ALU.subtract, op1=ALU.mult,
        )
        if trp is None:
            # bf16 xbar DMA transpose: no PE work, no PSUM bounce
            for d in range(DT):
                nc.sync.dma_start_transpose(
                    out=zt_out[:, d, tt * 128:(tt + 1) * 128],
                    in_=zn[:, d * 128:(d + 1) * 128],
                )
        else:
            for g in range(2):
                ps = trp.tile([128, 4, 128], BF16, tag="trps")
                for i in range(4):
                    nc.tensor.transpose(
                        ps[:, i, :],
                        zn[:, (4 * g + i) * 128:(4 * g + i + 1) * 128], ident
                    )
                cp = nc.scalar.copy if copies_on_act else nc.vector.tensor_copy
                cp(out=zt_out[:, 4 * g:4 * g + 4, tt * 128:(tt + 1) * 128],
                   in_=ps)


def build_nc():
    nc = bacc.Bacc("TRN2", target_bir_lowering=False, debug=False, num_devices=N_CORES)

    x = nc.dram_tensor("x", [T_ALL, D], FP32, kind="ExternalInput")
    wqkv = nc.dram_tensor("wqkv", [D, 3 * D], BF16, kind="ExternalInput")
    bqkv = nc.dram_tensor("bqkv", [3 * D], FP32, kind="ExternalInput")
    wproj = nc.dram_tensor("wproj", [D, D], BF16, kind="ExternalInput")
    bproj = nc.dram_tensor("bproj", [D], BF16, kind="ExternalInput")
    w1 = nc.dram_tensor("w1", [D, HID], BF16, kind="ExternalInput")
    b1 = nc.dram_tensor("b1", [HID], FP32, kind="ExternalInput")
    w2 = nc.dram_tensor("w2", [HID, D], BF16, kind="ExternalInput")
    b2 = nc.dram_tensor("b2", [D], BF16, kind="ExternalInput")
    y = nc.dram_tensor("y", [T_OWN, D], FP32, kind="ExternalOutput")
    wqkv_t = wqkv.ap().rearrange("(dt p) f -> p dt f", p=128)   # [128, 8, 3072]
    w1_t = w1.ap().rearrange("(dt p) f -> p dt f", p=128)       # [128, 8, 4096]
    w2_t = w2.ap().rearrange("(jt p) f -> p jt f", p=128)       # [128, 32, 1024]

    with tile.TileContext(nc) as tc, ExitStack() as ctx:
        P = ctx.enter_context

        # ---- whole-kernel pools ----
        singles = P(tc.tile_pool(name="singles", bufs=1))
        xpool = P(tc.tile_pool(name="xin", bufs=3))
        statpool = P(tc.tile_pool(name="stat", bufs=6))
        znpool = P(tc.tile_pool(name="zn", bufs=4))
        es_ao = ExitStack()
        aop = es_ao.enter_context(tc.tile_pool(name="aop", bufs=1, side="right"))
        es_pjw = ExitStack()
        pjw_pool = es_pjw.enter_context(
            tc.tile_pool(name="pjw", bufs=1, side="right"))

        # ---- constants ----
        ident = singles.tile([128, 128], BF16)
        make_identity(nc, ident)
        ones_bf = singles.tile([1, 128], BF16)
        nc.vector.memset(ones_bf, 1.0)
        eps_sb = singles.tile([128, 1], FP32)
        nc.vector.memset(eps_sb, EPS)
        bq_sb = singles.tile([128, 24], FP32)
        nc.sync.dma_start(out=bq_sb, in_=bqkv.ap().rearrange("(f p) -> p f", p=128))
        b1_sb = singles.tile([128, 32], FP32)
        nc.sync.dma_start(out=b1_sb, in_=b1.ap().rearrange("(f p) -> p f", p=128))
        bproj_sb = singles.tile([1, D], BF16)
        nc.sync.dma_start(out=bproj_sb, in_=bproj.ap().rearrange("(o f) -> o f", o=1))
        b2_sb = singles.tile([1, D], BF16)
        nc.sync.dma_start(out=b2_sb, in_=b2.ap().rearrange("(o f) -> o f", o=1))
        # V-bias broadcast to all partitions [128, 1024]
        vbias_sb = singles.tile([128, D], FP32)
        nc.sync.dma_start(
            out=vbias_sb,
            in_=bass.AP(tensor=bqkv, offset=2 * D, ap=[[0, 128], [1, D]]),
        )

        # ---- phase A: LN1 + transpose -> z1T ----
        es_z1 = ExitStack()
        z1p = es_z1.enter_context(tc.tile_pool(name="z1p", bufs=1, side="right"))
        z1T = z1p.tile([128, DT, T_ALL], BF16, tag="z1T")

        def load_x(tt):
            xt = xpool.tile([128, D], FP32, tag="xa")
            nc.sync.dma_start(out=xt, in_=x[tt * 128:(tt + 1) * 128, :])
            return xt

        with tc.tile_pool(name="psA", bufs=2, space="PSUM") as trpsA:
            _ln_transpose(nc, statpool, znpool, trpsA, load_x, T_ALL // 128,
                          z1T, eps_sb, ident, copies_on_act=True)

        # proj weights: prefetch now (used ~300us later in phase D)
        projw_sb = pjw_pool.tile([128, DT, D], BF16, tag="projw")
        nc.sync.dma_start(
            out=projw_sb, in_=wproj.ap().rearrange("(dt p) f -> p dt f", p=128)
        )

        # ---- fused QKV + attention ----
        es_kqv = ExitStack()
        kqvp = es_kqv.enter_context(tc.tile_pool(name="kqvp", bufs=1))
        kt_all = kqvp.tile([128, DT, T_ALL], BF16, tag="kt")
        qt_all = kqvp.tile([128, DT, T_OWN], BF16, tag="qt")
        VP = kqvp.tile([128, 16, 16 * (HD + 1)], BF16, tag="vp")
        vp_ones = VP.rearrange("p k (h e) -> p k h e", e=HD + 1)[:, :, :, HD:HD + 1]
        nc.vector.memset(vp_ones, 1.0)
        aoT = aop.tile([128, DT, T_OWN], BF16, tag="aoT")

        with (
            tc.tile_pool(name="wq", bufs=2, side="right") as wq_pool,
            tc.tile_pool(name="wv", bufs=1, side="right") as wv_pool,
            tc.tile_pool(name="psB", bufs=2, space="PSUM") as qkpsum,
            tc.tile_pool(name="exps", bufs=4) as exp_pool,
            tc.tile_pool(name="nrm", bufs=2) as nrm_pool,
            tc.tile_pool(name="psCs", bufs=2, space="PSUM") as spsum,
            tc.tile_pool(name="psCa", bufs=2, space="PSUM") as avpsum,
            tc.tile_pool(name="drp", bufs=3, space="DRAM") as drpool,
        ):
            def kq_fillers(j):
                out = []
                state = {}
                for f in (8 + j, j):
                    nch = 4 if f >= 8 else 2
                    for tcn in range(nch):
                        def grp(f=f, tcn=tcn):
                            if f not in state:
                                wq_f = wq_pool.tile([128, DT, 128], BF16,
                                                    tag="wqf")
                                nc.sync.dma_start(
                                    out=wq_f,
                                    in_=wqkv_t[:, :, f * 128:(f + 1) * 128])
                                state[f] = wq_f
                            wq_f = state[f]
                            ps = qkpsum.tile([128, 512], FP32, tag="qkps")
                            for d in range(DT):
                                nc.tensor.matmul(
                                    ps, wq_f[:, d, :],
                                    z1T[:, d, tcn * 512:(tcn + 1) * 512],
                                    start=(d == 0), stop=(d == DT - 1),
                                )
                            if f >= 8:
                                dst = kt_all[:, f - 8, tcn * 512:(tcn + 1) * 512]
                            else:
                                dst = qt_all[:, f, tcn * 512:(tcn + 1) * 512]
                            nc.vector.tensor_scalar(
                                out=dst, in0=ps, scalar1=bq_sb[:, f:f + 1],
                                scalar2=None, op0=ALU.add,
                            )
                        out.append(grp)
                return out

            def v_fillers(vc):
                out = []
                state = {}
                for tt in range(T_ALL // 128):
                    def grp(tt=tt):
                        if "wv" not in state:
                            wv = wv_pool.tile([128, DT, 512], BF16, tag="wvf")
                            nc.sync.dma_start(
                                out=wv,
                                in_=wqkv_t[:, :, 2 * D + vc * 512:
                                           2 * D + (vc + 1) * 512])
                            state["wv"] = wv
                        wv = state["wv"]
                        ps = qkpsum.tile([128, 512], FP32, tag="qkps")
                        for d in range(DT):
                            nc.tensor.matmul(
                                ps, z1T[:, d, tt * 128:(tt + 1) * 128],
                                wv[:, d, :],
                                start=(d == 0), stop=(d == DT - 1),
                            )
                        dst = VP[:, tt, vc * 8 * (HD + 1):
                                 (vc + 1) * 8 * (HD + 1)]
                        dst = dst.rearrange(
                            "p (h e) -> p h e", e=HD + 1)[:, :, 0:HD]
                        srcp = ps.rearrange("p (h e) -> p h e", e=HD)
                        vb = vbias_sb[:, vc * 512:(vc + 1) * 512].rearrange(
                            "p (h e) -> p h e", e=HD)
                        nc.vector.scalar_tensor_tensor(
                            out=dst, in0=srcp, scalar=0.0, in1=vb,
                            op0=ALU.bypass, op1=ALU.add,
                        )
                    out.append(grp)
                return out

            def emit_pair_fill(j, fillers, per_kt=None):
                """Process head pair (2j, 2j+1) with scores row-packed into
                the two 64-row halves of the PE array (tile_position), one
                q-chunk at a time so PSUM stays within 8 banks. Filler
                psum-groups are popped every 3rd kt step."""
                h0, h1 = 2 * j, 2 * j + 1
                for qc in range(2):
                    avs = []
                    for hh in (h0, h1):
                        av_t = avpsum.tile([HD + 1, 512], FP32, tag="av")
                        avs.append(av_t)
                    for kt in range(T_ALL // 128):
                        sp = spsum.tile([128, T_OWN], FP32, tag="sps")
                        for hi, hh in enumerate((h0, h1)):
                            pr = hi * 64
                            nc.tensor.matmul(
                                sp[:, hi * 512:(hi + 1) * 512],
                                kt_all[pr:pr + 64, j, kt * 128:(kt + 1) * 128],
                                qt_all[pr:pr + 64, j, qc * 512:(qc + 1) * 512],
                                start=True, stop=True,
                                tile_position=(pr, 0),
                            )
                        ex = exp_pool.tile([128, T_OWN], BF16, tag="exp")
                        nc.scalar.activation(out=ex, in_=sp, func=AF.Exp,
                                             scale=0.125)
                        if per_kt is not None and qc == 0:
                            per_kt(kt)
                        for hi, hh in enumerate((h0, h1)):
                            nc.tensor.matmul(
                                avs[hi],
                                VP[:, kt, hh * (HD + 1):(hh + 1) * (HD + 1)],
                                ex[:, hi * 512:(hi + 1) * 512],
                                start=(kt == 0), stop=(kt == T_ALL // 128 - 1),
                            )
                        if kt % 3 == 2 and fillers:
                            fillers.popleft()()
                    for hi, hh in enumerate((h0, h1)):
                        av = avs[hi]
                        ft, pr = hh // 2, (hh % 2) * 64
                        asl = aoT[pr:pr + 64, ft, qc * 512:(qc + 1) * 512]
                        nc.vector.tensor_copy(out=asl, in_=av[0:HD, :])
                        sums_sb = nrm_pool.tile([1, 512], FP32, tag="sums")
                        nc.vector.tensor_copy(out=sums_sb, in_=av[HD:HD + 1, :])
                        rec = nrm_pool.tile([1, 512], FP32, tag="rec")
                        nc.vector.reciprocal_approx_fast(out=rec, in_=sums_sb)
                        rec_bf = nrm_pool.tile([1, 512], BF16, tag="recbf")
                        nc.vector.tensor_copy(out=rec_bf, in_=rec)
                        drt = drpool.tile([1, 512], BF16, tag="drrec")
                        nc.sync.dma_start(out=drt, in_=rec_bf)
                        bcs = nrm_pool.tile([128, 512], BF16, tag="bcs")
                        nc.sync.dma_start(out=bcs,
                                          in_=drt.broadcast_to([128, 512]))
                        nc.vector.tensor_mul(
                            out=asl, in0=asl, in1=bcs[pr:pr + HD, :]
                        )

            vf1 = v_fillers(1)
            for f in kq_fillers(0):
                f()
            v0 = v_fillers(0)
            for j in range(8):
                fillers = deque()
                if j + 1 < 8:
                    fillers.extend(kq_fillers(j + 1))
                if j < 4:
                    fillers.extend(vf1[j * 4:(j + 1) * 4])
                if j == 0:
                    # pair 0 drives V(vc0, kt) just-in-time for its attnV
                    emit_pair_fill(0, fillers, per_kt=lambda kt: v0[kt]())
                else:
                    emit_pair_fill(j, fillers)
                # flush leftover fillers before next pair needs them
                while fillers:
                    fillers.popleft()()
        es_z1.close()  # z1T dead
        es_kqv.close()  # kt/qt/VP dead

        # ---- phase D: proj + residual -> x2 ----
        es_x2 = ExitStack()
        x2p = es_x2.enter_context(tc.tile_pool(name="x2p", bufs=1))
        x2_all = x2p.tile([128, T_OWN // 128, D], FP32, tag="x2")
        with tc.tile_pool(name="psD", bufs=2, space="PSUM") as ppsum:
            for tt in range(T_OWN // 128):
                xo = xpool.tile([128, D], FP32, tag="xa")
                nc.sync.dma_start(out=xo, in_=x[tt * 128:(tt + 1) * 128, :])
                for oc in range(2):
                    ps = ppsum.tile([128, 512], FP32, tag="pps")
                    for d in range(DT):
                        nc.tensor.matmul(
                            ps, aoT[:, d, tt * 128:(tt + 1) * 128],
                            projw_sb[:, d, oc * 512:(oc + 1) * 512],
                            start=(d == 0), stop=False,
                        )
                    nc.tensor.matmul(
                        ps, ones_bf, bproj_sb[:, oc * 512:(oc + 1) * 512],
                        start=False, stop=True,
                    )
                    nc.vector.scalar_tensor_tensor(
                        out=x2_all[:, tt, oc * 512:(oc + 1) * 512],
                        in0=ps, scalar=0.0, in1=xo[:, oc * 512:(oc + 1) * 512],
                        op0=ALU.bypass, op1=ALU.add,
                    )
        es_pjw.close()
        es_ao.close()  # aoT dead

        # ---- phase E: LN2 + transpose -> z2T ----
        es_z2 = ExitStack()
        z2p = es_z2.enter_context(tc.tile_pool(name="z2p", bufs=1))
        z2T = z2p.tile([128, DT, T_OWN], BF16, tag="z2T")
        with tc.tile_pool(name="psE", bufs=2, space="PSUM") as trpsE:
            _ln_transpose(nc, statpool, znpool, trpsE,
                          lambda tt: x2_all[:, tt, :], T_OWN // 128,
                          z2T, eps_sb, ident)

        # ---- phase F: MLP ----
        with (
            tc.tile_pool(name="yp", bufs=3) as ypool,
            tc.tile_pool(name="w1p", bufs=3) as w1_pool,
            tc.tile_pool(name="w2p", bufs=1) as w2_pool,
            tc.tile_pool(name="hp", bufs=1) as hpool,
            tc.tile_pool(name="psF", bufs=6, space="PSUM") as fpsum,
        ):
            w2_sb = w2_pool.tile([128, HID // 128, D], BF16, tag="w2sb")
            nc.sync.dma_start(out=w2_sb, in_=w2_t)
            for tc2 in range(2):
                hT = hpool.tile([128, HID // 128, 512], BF16, tag="hT")
                for jt in range(HID // 128):
                    w1f = w1_pool.tile([128, DT, 128], BF16, tag="w1f")
                    nc.sync.dma_start(
                        out=w1f, in_=w1_t[:, :, jt * 128:(jt + 1) * 128]
                    )
                    ps = fpsum.tile([128, 512], FP32, tag="fps")
                    for d in range(DT):
                        nc.tensor.matmul(
                            ps, w1f[:, d, :], z2T[:, d, tc2 * 512:(tc2 + 1) * 512],
                            start=(d == 0), stop=(d == DT - 1),
                        )
                    nc.scalar.activation(
                        out=hT[:, jt, :], in_=ps, func=AF.Gelu,
                        bias=b1_sb[:, jt:jt + 1],
                    )
                for oc in range(2):
                    pss = []
                    for i in range(4):
                        ops_t = fpsum.tile([128, 512], FP32, tag="fps")
                        pss.append(ops_t)
                    for jt in range(HID // 128):
                        for tt in range(4):
                            nc.tensor.matmul(
                                pss[tt], hT[:, jt, tt * 128:(tt + 1) * 128],
                                w2_sb[:, jt, oc * 512:(oc + 1) * 512],
                                start=(jt == 0), stop=False,
                            )
                    for tt in range(4):
                        tglob = tc2 * 4 + tt
                        nc.tensor.matmul(
                            pss[tt], ones_bf, b2_sb[:, oc * 512:(oc + 1) * 512],
                            start=False, stop=True,
                        )
                        ys = ypool.tile([128, 512], FP32, tag="ys")
                        nc.vector.scalar_tensor_tensor(
                            out=ys, in0=pss[tt], scalar=0.0,
                            in1=x2_all[:, tglob, oc * 512:(oc + 1) * 512],
                            op0=ALU.bypass, op1=ALU.add,
                        )
                        nc.sync.dma_start(
                            out=y[tglob * 128:(tglob + 1) * 128,
                                  oc * 512:(oc + 1) * 512],
                            in_=ys,
                        )
        es_z2.close()
        es_x2.close()

    nc.compile()
    return nc


def prep_host_inputs(inputs):
    """Fold LN affine params into the adjacent matmul weights, cast to bf16,
    and build the 8 per-core input maps."""
    import ml_dtypes

    f32 = np.float32
    x = np.asarray(inputs["x"], f32)
    qkv_w = np.asarray(inputs["qkv_w"], f32)
    qkv_b = np.asarray(inputs["qkv_b"], f32)
    proj_w = np.asarray(inputs["proj_w"], f32)
    proj_b = np.asarray(inputs["proj_b"], f32)
    fc1_w = np.asarray(inputs["fc1_w"], f32)
    fc1_b = np.asarray(inputs["fc1_b"], f32)
    fc2_w = np.asarray(inputs["fc2_w"], f32)
    fc2_b = np.asarray(inputs["fc2_b"], f32)
    ln1_w = np.asarray(inputs["ln1_w"], f32)
    ln1_b = np.asarray(inputs["ln1_b"], f32)
    ln2_w = np.asarray(inputs["ln2_w"], f32)
    ln2_b = np.asarray(inputs["ln2_b"], f32)

    bf = ml_dtypes.bfloat16
    wqkv = (ln1_w[:, None] * qkv_w).astype(bf)
    bqkv = (qkv_b + ln1_b @ qkv_w).astype(f32)
    w1 = (ln2_w[:, None] * fc1_w).astype(bf)
    b1 = (fc1_b + ln2_b @ fc1_w).astype(f32)

    shared = {
        "wqkv": wqkv, "bqkv": bqkv,
        "wproj": proj_w.astype(bf), "bproj": proj_b.astype(bf),
        "w1": w1, "b1": b1,
        "w2": fc2_w.astype(bf), "b2": fc2_b.astype(bf),
    }
    in_maps = []
    for c in range(N_CORES):
        b, half = c // 2, c % 2
        own = x[b, half * 1024:(half + 1) * 1024]
        other = x[b, (1 - half) * 1024:(2 - half) * 1024]
        xc = np.concatenate([own, other], axis=0)
        in_maps.append({"x": np.ascontiguousarray(xc), **shared})
    return in_maps


# ---------------------------------------------------------------------------
# Cached PJRT runner (jit once, reuse across kernel() calls)
# ---------------------------------------------------------------------------
_CACHE = {}


def _get_runner():
    if "runner" in _CACHE:
        return _CACHE["runner"]

    from jax.experimental.shard_map import shard_map
    from jax.sharding import Mesh, PartitionSpec
    from concourse.bass2jax import (
        _bass_exec_p, install_neuronx_cc_hook, partition_id_tensor,
    )

    nc = build_nc()
    install_neuronx_cc_hook()

    partition_name = nc.partition_id_tensor.name if nc.partition_id_tensor else None
    in_names, out_names, out_avals, zero_shapes = [], [], [], []
    for alloc in nc.m.functions[0].allocations:
        if not isinstance(alloc, mybir.MemoryLocationSet):
            continue
        name = alloc.memorylocations[0].name
        if alloc.kind == "ExternalInput":
            if name != partition_name:
                in_names.append(name)
        elif alloc.kind == "ExternalOutput":
            shape = tuple(alloc.tensor_shape)
            dtype = mybir.dt.np(alloc.dtype)
            out_names.append(name)
            out_avals.append(jax.core.ShapedArray(shape, dtype))
            zero_shapes.append((shape, dtype))
    n_params = len(in_names)
    n_outs = len(out_names)
    all_in = list(in_names) + list(out_names)
    if partition_name is not None:
        all_in.append(partition_name)
    donate = tuple(range(n_params, n_params + n_outs))

    def _body(*args):
        operands = list(args)
        if partition_name is not None:
            operands.append(partition_id_tensor())
        outs = _bass_exec_p.bind(
            *operands,
            out_avals=tuple(out_avals),
            in_names=tuple(all_in),
            out_names=tuple(out_names),
            lowering_input_output_aliases=(),
            sim_require_finite=True,
            sim_require_nnan=True,
            nc=nc,
        )
        return tuple(outs)

    devices = jax.devices()[:N_CORES]
    mesh = Mesh(np.asarray(devices), ("core",))
    sharded = jax.jit(
        shard_map(
            _body, mesh=mesh,
            in_specs=(PartitionSpec("core"),) * (n_params + n_outs),
            out_specs=(PartitionSpec("core"),) * n_outs,
            check_rep=False,
        ),
        donate_argnums=donate, keep_unused=True,
    )

    def run(in_maps):
        concat_in = [
            np.concatenate([np.asarray(m[name]) for m in in_maps], axis=0)
            for name in in_names
        ]
        concat_zeros = [
            np.zeros((N_CORES * s[0], *s[1:]), dt) for (s, dt) in zero_shapes
        ]
        out_arrs = sharded(*concat_in, *concat_zeros)
        per_core = []
        for c in range(N_CORES):
            per_core.append({
                name: np.asarray(out_arrs[i]).reshape(
                    N_CORES, *out_avals[i].shape)[c]
                for i, name in enumerate(out_names)
            })
        return per_core

    _CACHE["runner"] = run
    return run


def kernel(**inputs) -> np.ndarray:
    run = _get_runner()
    in_maps = prep_host_inputs(inputs)
    results = run(in_maps)
    out = np.zeros((4, 2048, 1024), np.float32)
    for c in range(N_CORES):
        b, half = c // 2, c % 2
        out[b, half * 1024:(half + 1) * 1024] = results[c]["y"]
    return out



# revision 43
# speedup vs baseline: 1.2835x; 1.2835x over previous
"""Transformer block (pre-norm attention + MLP) on 8 TRN2 NeuronCores.

Sharding: 8 cores = 4 batch elements x 2 sequence halves (data parallel, no
collectives). Each core computes its 1024 "own" query tokens end-to-end and
redundantly builds K/V for the full 2048-token batch element. The k-token
order is permuted per core (own tokens first) so the SPMD program is
identical on every core - softmax over k is permutation invariant.

v2: fp8e4 + DoubleRow matmuls for QKV / attnV / MLP (2 contraction tiles per
instruction at 0.5 cycles/row), attnV emitted in [q-partition, head-dim]
orientation (full 128-wide PE output), LayerNorm applied on the scalar
engine (Identity activation with per-partition scale/bias), transposes on
PE with psum drains split between DVE and scalar engines, ao transposed
back via the DMA transpose engine. Scores stay bf16 (64-deep contraction
cannot use DoubleRow). Residual path stays fp32. LayerNorm affine params
are folded into adjacent matmul weights host-side; softmax denominators
come free from an appended ones-column on V.
"""

from collections import deque
from contextlib import ExitStack

import numpy as np

try:
    import jax
    jax.config.update("jax_compilation_cache_dir", "/tmp/jax_bass_cache")
    jax.config.update("jax_persistent_cache_min_compile_time_secs", 0.0)
    jax.config.update("jax_persistent_cache_min_entry_size_bytes", -1)
except Exception:
    import jax

import concourse.bacc as bacc
import concourse.bass as bass
import concourse.mybir as mybir
import concourse.tile as tile
from concourse.masks import make_identity

FP32 = mybir.dt.float32
BF16 = mybir.dt.bfloat16
FP8 = mybir.dt.float8e4
AF = mybir.ActivationFunctionType
ALU = mybir.AluOpType
PM = mybir.MatmulPerfMode

D = 1024          # model dim
DT = 8            # d tiles of 128
DP = 4            # d tile pairs (DoubleRow)
H = 16            # heads
HD = 64           # head dim
HID = 4096        # mlp hidden
JT = 32           # hidden tiles of 128
JP = 16           # hidden tile pairs
T_ALL = 2048      # tokens per core incl. K/V-only tokens
T_OWN = 1024      # query/output tokens per core
EPS = 1e-6
N_CORES = 8


def build_nc(with_bias=False, f8_attn=True, debug_out=False):
    nc = bacc.Bacc("TRN2", target_bir_lowering=False, debug=False,
                   num_devices=N_CORES)

    w8 = FP8 if f8_attn else BF16
    x = nc.dram_tensor("x", [T_ALL, D], FP32, kind="ExternalInput")
    wqkv = nc.dram_tensor("wqkv", [D, 3 * D], w8, kind="ExternalInput")
    bqkv = nc.dram_tensor("bqkv", [3 * D], FP32, kind="ExternalInput")
    wproj = nc.dram_tensor("wproj", [D, D], BF16, kind="ExternalInput")
    # fc1/fc2 weights scaled x32/x64 host-side and split into fp8 base +
    # fp8 residual (raw fc weights sit in e4m3's subnormal range)
    w1a = nc.dram_tensor("w1a", [D, HID], FP8, kind="ExternalInput")
    w1b = nc.dram_tensor("w1b", [D, HID], FP8, kind="ExternalInput")
    b1 = nc.dram_tensor("b1", [HID], FP32, kind="ExternalInput")
    w2a = nc.dram_tensor("w2a", [HID, D], FP8, kind="ExternalInput")
    w2b = nc.dram_tensor("w2b", [HID, D], FP8, kind="ExternalInput")
    if with_bias:
        bproj = nc.dram_tensor("bproj", [D], BF16, kind="ExternalInput")
        b2 = nc.dram_tensor("b2", [D], BF16, kind="ExternalInput")
    y = nc.dram_tensor("y", [T_OWN, D], FP32, kind="ExternalOutput")
    if debug_out:
        kto = nc.dram_tensor("kto", [128, DT, T_ALL], BF16, kind="ExternalOutput")
        qto = nc.dram_tensor("qto", [128, DT, T_OWN], BF16, kind="ExternalOutput")
        vpo = nc.dram_tensor("vpo", [128, 16, 16 * (HD + 1)],
                             w8, kind="ExternalOutput")
        aoto = nc.dram_tensor("aoto", [128, DT, T_OWN], BF16, kind="ExternalOutput")
        x2o = nc.dram_tensor("x2o", [128, T_OWN // 128, D], FP32, kind="ExternalOutput")
    wqkv_t = wqkv.ap().rearrange("(dt p) f -> p dt f", p=128)   # [128, 8, 3072]
    w1a_t = w1a.ap().rearrange("(dt p) f -> p dt f", p=128)     # [128, 8, 4096]
    w1b_t = w1b.ap().rearrange("(dt p) f -> p dt f", p=128)
    w2a_t = w2a.ap().rearrange("(jt p) f -> p jt f", p=128)     # [128, 32, 1024]
    w2b_t = w2b.ap().rearrange("(jt p) f -> p jt f", p=128)
    wproj_t = wproj.ap().rearrange("(dt p) f -> p dt f", p=128)

    with tile.TileContext(nc) as tc, ExitStack() as ctx:
        P = ctx.enter_context

        # ---- whole-kernel pools ----
        singles = P(tc.tile_pool(name="singles", bufs=1))
        xpool = P(tc.tile_pool(name="xin", bufs=2))
        statpool = P(tc.tile_pool(name="stat", bufs=8))
        znpool = P(tc.tile_pool(name="zn", bufs=3))
        es_ao = ExitStack()
        aop = es_ao.enter_context(tc.tile_pool(name="aop", bufs=1, side="right"))
        es_pjw, es_w1a, es_w1b = ExitStack(), ExitStack(), ExitStack()

        # ---- constants ----
        ident = singles.tile([128, 128], BF16)
        make_identity(nc, ident)
        eps_sb = singles.tile([128, 1], FP32)
        nc.vector.memset(eps_sb, EPS)
        nexp_sb = singles.tile([128, 1], FP32)
        nc.vector.memset(nexp_sb, -4.0)
        bq_sb = singles.tile([128, 24], FP32)
        nc.sync.dma_start(out=bq_sb, in_=bqkv.ap().rearrange("(f p) -> p f", p=128))
        b1_sb = singles.tile([128, 32], FP32)
        nc.sync.dma_start(out=b1_sb, in_=b1.ap().rearrange("(f p) -> p f", p=128))
        # V-bias broadcast to all partitions [128, 1024]
        vbias_sb = singles.tile([128, D], FP32)
        nc.sync.dma_start(
            out=vbias_sb,
            in_=bass.AP(tensor=bqkv, offset=2 * D, ap=[[0, 128], [1, D]]),
        )
        if with_bias:
            ones_bf = singles.tile([1, 128], BF16)
            nc.vector.memset(ones_bf, 1.0)
            bproj_sb = singles.tile([1, D], BF16)
            nc.sync.dma_start(out=bproj_sb,
                              in_=bproj.ap().rearrange("(o f) -> o f", o=1))
            b2_sb = singles.tile([1, D], BF16)
            nc.sync.dma_start(out=b2_sb,
                              in_=b2.ap().rearrange("(o f) -> o f", o=1))

        def ln_tile(xt, trp, dst, tt, res_dst=None):
            """LayerNorm one [128, D] fp32 tile -> transposed dst[:, :, tt*128]
            (dst dtype may be fp8/bf16). Stats+drain-g0 on DVE, sqrt+apply+
            drain-g1 on scalar engine, transposes on PE."""
            stats = statpool.tile([128, 2, 6], FP32, tag="stats")
            nc.vector.bn_stats(out=stats[:, 0, :], in_=xt[:, 0:512])
            nc.vector.bn_stats(out=stats[:, 1, :], in_=xt[:, 512:1024])
            mv = statpool.tile([128, 2], FP32, tag="mv")
            nc.vector.bn_aggr(out=mv, in_=stats)
            sd = statpool.tile([128, 1], FP32, tag="sd")
            nc.scalar.activation(out=sd, in_=mv[:, 1:2], func=AF.Sqrt,
                                 bias=eps_sb)
            rinv = statpool.tile([128, 1], FP32, tag="rinv")
            nc.vector.reciprocal(out=rinv, in_=sd)
            zn = znpool.tile([128, D], BF16, tag="zn")
            nc.gpsimd.tensor_scalar(
                out=zn, in0=xt, scalar1=mv[:, 0:1], scalar2=rinv,
                op0=ALU.subtract, op1=ALU.mult,
            )
            for g in range(2):
                ps = trp.tile([128, 4, 128], BF16, tag="trps")
                for i in range(4):
                    nc.tensor.transpose(
                        ps[:, i, :],
                        zn[:, (4 * g + i) * 128:(4 * g + i + 1) * 128], ident)
                dstsl = dst[:, 4 * g:4 * g + 4, tt * 128:(tt + 1) * 128]
                if g == 0:
                    nc.vector.tensor_copy(out=dstsl, in_=ps)
                else:
                    nc.scalar.copy(out=dstsl, in_=ps)
                if res_dst is not None:
                    ressl = res_dst[:, 4 * g:4 * g + 4, tt * 128:(tt + 1) * 128]
                    nc.vector.tensor_tensor(out=ressl, in0=ps, in1=dstsl,
                                            op=ALU.subtract)

        # ---- persistent attention tensors ----
        es_z1 = ExitStack()
        z1p = es_z1.enter_context(tc.tile_pool(name="z1p", bufs=1, side="right"))
        z1T = z1p.tile([128, DT, T_ALL], w8, tag="z1T")

        es_kqv = ExitStack()
        kqvp = es_kqv.enter_context(tc.tile_pool(name="kqvp", bufs=1))
        kt_all = kqvp.tile([128, DT, T_ALL], BF16, tag="kt")
        qt_all = kqvp.tile([128, DT, T_OWN], BF16, tag="qt")
        VP = kqvp.tile([128, 16, 16 * (HD + 1)], w8, tag="vp")
        vp_ones = VP.rearrange("p k (h e) -> p k h e", e=HD + 1)[:, :, :, HD:HD + 1]
        nc.vector.memset(vp_ones, 1.0)

        aoT = aop.tile([128, DT, T_OWN], BF16, tag="aoT")

        # ---- filler machinery: QKV compute groups ----
        es_wq = ExitStack()
        wq_pool = es_wq.enter_context(tc.tile_pool(name="wq", bufs=2, side="right"))
        wv_pool = es_wq.enter_context(tc.tile_pool(name="wv", bufs=1, side="right"))
        es_qkps = ExitStack()
        qkpsum = es_qkps.enter_context(
            tc.tile_pool(name="psB", bufs=2, space="PSUM"))
        wq_state, wv_state = {}, {}

        def qkv_matmuls(ps, lhsT_w, rhs_z, reverse=False):
            """contract 1024 into ps [128, N]; operands [128, DT, *]."""
            if f8_attn:
                for dp in range(DP):
                    a_ = lhsT_w[:, 2 * dp:2 * dp + 2, :]
                    b_ = rhs_z[:, 2 * dp:2 * dp + 2, :]
                    nc.tensor.matmul(
                        ps, b_ if reverse else a_, a_ if reverse else b_,
                        start=(dp == 0), stop=(dp == DP - 1),
                        perf_mode=PM.DoubleRow)
            else:
                for d in range(DT):
                    a_ = lhsT_w[:, d, :]
                    b_ = rhs_z[:, d, :]
                    nc.tensor.matmul(
                        ps, b_ if reverse else a_, a_ if reverse else b_,
                        start=(d == 0), stop=(d == DT - 1))

        def kq_group(f, tcn):
            """K (f in 8..15) or Q (f in 0..7) for token chunk tcn*512."""
            if f not in wq_state:
                wq_f = wq_pool.tile([128, DT, 128], w8, tag="wqf")
                nc.sync.dma_start(out=wq_f,
                                  in_=wqkv_t[:, :, f * 128:(f + 1) * 128])
                wq_state[f] = wq_f
            wq_f = wq_state[f]
            ps = qkpsum.tile([128, 512], FP32, tag="qkps")
            qkv_matmuls(ps, wq_f, z1T[:, :, tcn * 512:(tcn + 1) * 512])
            if f >= 8:
                dst = kt_all[:, f - 8, tcn * 512:(tcn + 1) * 512]
            else:
                dst = qt_all[:, f, tcn * 512:(tcn + 1) * 512]
            nc.vector.tensor_scalar(
                out=dst, in0=ps, scalar1=bq_sb[:, f:f + 1],
                scalar2=None, op0=ALU.add,
            )

        def v_group(vc, tt):
            """V chunk vc (8 heads) for token tile tt."""
            if vc not in wv_state:
                wv = wv_pool.tile([128, DT, 512], w8, tag="wvf")
                nc.sync.dma_start(
                    out=wv,
                    in_=wqkv_t[:, :, 2 * D + vc * 512:2 * D + (vc + 1) * 512])
                wv_state[vc] = wv
            wv = wv_state[vc]
            ps = qkpsum.tile([128, 512], FP32, tag="qkps")
            qkv_matmuls(ps, wv, z1T[:, :, tt * 128:(tt + 1) * 128], reverse=True)
            dst = VP[:, tt, vc * 8 * (HD + 1):(vc + 1) * 8 * (HD + 1)]
            dst = dst.rearrange("p (h e) -> p h e", e=HD + 1)[:, :, 0:HD]
            srcp = ps.rearrange("p (h e) -> p h e", e=HD)
            vb = vbias_sb[:, vc * 512:(vc + 1) * 512].rearrange(
                "p (h e) -> p h e", e=HD)
            nc.vector.scalar_tensor_tensor(
                out=dst, in0=srcp, scalar=0.0, in1=vb,
                op0=ALU.bypass, op1=ALU.add,
            )

        # ---- filler schedule: QKV groups emitted on demand / paced ----
        # f-major order: pair-0-critical groups (K f=8, Q f=0) first.
        forder = (8, 0, 9, 1, 10, 2, 11, 3, 12, 4, 13, 5, 14, 6, 15, 7)
        order, bodies, ready_chunk = [], {}, {}
        for f in forder:
            for tcn in range(4 if f >= 8 else 2):
                key = ("kq", f, tcn)
                order.append(key)
                bodies[key] = lambda f=f, tcn=tcn: kq_group(f, tcn)
                ready_chunk[key] = tcn
        for tt in range(16):
            key = ("v1", tt)
            order.append(key)
            bodies[key] = lambda tt=tt: v_group(1, tt)
            ready_chunk[key] = tt // 4
        emitted = set()

        def emit_key(key):
            if key not in emitted:
                emitted.add(key)
                bodies[key]()

        def pace(max_n, chunk_ready=99, loaded_only=False):
            n = 0
            for key in order:
                if n >= max_n:
                    break
                if loaded_only and not (key[0] == "kq" and key[1] in wq_state):
                    continue
                if key not in emitted and ready_chunk[key] <= chunk_ready:
                    emit_key(key)
                    n += 1

        def pending():
            return any(key not in emitted for key in order)

        # ---- phase A: LN1 + transpose -> z1T, with early fillers ----
        with tc.tile_pool(name="psA", bufs=2, space="PSUM") as trpsA:
            for tt in range(T_ALL // 128):
                xt = xpool.tile([128, D], FP32, tag="xa")
                nc.sync.dma_start(out=xt, in_=x[tt * 128:(tt + 1) * 128, :])
                ln_tile(xt, trpsA, z1T, tt)
                if tt >= 4 and False:
                    pace(2, (tt + 1) // 4 - 1, loaded_only=True)

        # ---- fused attention ----
        with (
            tc.tile_pool(name="sps", bufs=2, space="PSUM") as spsum,
            tc.tile_pool(name="avps", bufs=1, space="PSUM") as avpsum,
            tc.tile_pool(name="exps", bufs=2) as exp_pool,
            tc.tile_pool(name="aos", bufs=2) as aos_pool,
            tc.tile_pool(name="nrm", bufs=2) as nrm_pool,
        ):
            def emit_pair(j, qc, per_kt=None):
                h0 = 2 * j
                av_a = avpsum.tile([128, 4, HD + 1], FP32, tag="av0")
                av_b = avpsum.tile([128, 4, HD + 1], FP32, tag="av1")
                avs = [av_a, av_b]
                ex_all = exp_pool.tile([128, 16, T_OWN],
                                       FP8 if f8_attn else BF16, tag="exall")
                for kt in range(16):
                    sp = spsum.tile([128, T_OWN], FP32, tag="sps")
                    for hi in range(2):
                        pr = hi * 64
                        nc.tensor.matmul(
                            sp[:, hi * 512:(hi + 1) * 512],
                            kt_all[pr:pr + 64, j, kt * 128:(kt + 1) * 128],
                            qt_all[pr:pr + 64, j, qc * 512:(qc + 1) * 512],
                            start=True, stop=True,
                            tile_position=(pr, 0),
                        )
                    if per_kt is not None and qc == 0:
                        per_kt(kt)
                    # bias -4: keeps exp within fp8e4 range (max scaled
                    # score ~8 -> e^4 = 55 << 448); softmax denominator
                    # absorbs the uniform shift exactly
                    nc.scalar.activation(out=ex_all[:, kt, :], in_=sp,
                                         func=AF.Exp, scale=0.125,
                                         bias=nexp_sb)
                    if kt % 3 == 2:
                        pace(1)
                # attnV: sequential psum accumulation groups (hardware
                # corrupts interleaved DoubleRow accumulation groups)
                for hi in range(2):
                    for qi in range(4):
                        qsl = slice(hi * 512 + qi * 128,
                                    hi * 512 + (qi + 1) * 128)
                        hsl = slice((h0 + hi) * (HD + 1),
                                    (h0 + hi + 1) * (HD + 1))
                        if f8_attn:
                            for ktp in range(8):
                                nc.tensor.matmul(
                                    avs[hi][:, qi, :],
                                    ex_all[:, 2 * ktp:2 * ktp + 2, qsl],
                                    VP[:, 2 * ktp:2 * ktp + 2, hsl],
                                    start=(ktp == 0), stop=(ktp == 7),
                                    perf_mode=PM.DoubleRow,
                                )
                        else:
                            for kt in range(16):
                                nc.tensor.matmul(
                                    avs[hi][:, qi, :],
                                    ex_all[:, kt, qsl],
                                    VP[:, kt, hsl],
                                    start=(kt == 0), stop=(kt == 15),
                                )
                # normalize by softmax denominator (ones-column) + emit
                aos = aos_pool.tile([128, 4, 128], BF16, tag="aos")
                for hi in range(2):
                    rin = nrm_pool.tile([128, 4, 1], FP32, tag="rin")
                    nc.vector.reciprocal(out=rin, in_=avs[hi][:, :, HD:HD + 1])
                    for qi in range(4):
                        nc.vector.tensor_scalar(
                            out=aos[:, qi, hi * 64:(hi + 1) * 64],
                            in0=avs[hi][:, qi, 0:HD],
                            scalar1=rin[:, qi, :], scalar2=None, op0=ALU.mult,
                        )
                for qi in range(4):
                    nc.sync.dma_start_transpose(
                        out=aoT[:, j, (qc * 4 + qi) * 128:(qc * 4 + qi + 1) * 128],
                        in_=aos[:, qi, :])

            def ensure_pair_inputs(j):
                for tcn in range(4):
                    emit_key(("kq", 8 + j, tcn))
                for tcn in range(2):
                    emit_key(("kq", j, tcn))
                if j >= 4:
                    for tt in range(16):
                        emit_key(("v1", tt))

            for j in range(8):
                # pair inputs MUST be emitted before its score matmuls --
                # same-engine program order cannot be fixed by semaphores
                ensure_pair_inputs(j)
                if j == 0:
                    emit_pair(0, 0, per_kt=lambda kt: v_group(0, kt))
                    emit_pair(0, 1)
                else:
                    emit_pair(j, 0)
                    emit_pair(j, 1)
                if j == 3:
                    # all z1T consumers must be emitted before pools close
                    for key in order:
                        emit_key(key)
                if j == 4:
                    es_wq.close()
                    es_z1.close()   # z1T dead (all QKV groups emitted)
                    w1a_pool = es_w1a.enter_context(
                        tc.tile_pool(name="w1ap", bufs=1, side="right"))
                    w1a_sb = w1a_pool.tile([128, DT, HID], FP8, tag="w1asb")
                    nc.sync.dma_start(out=w1a_sb, in_=w1a_t)
                if j == 6:
                    pjw_pool = es_pjw.enter_context(
                        tc.tile_pool(name="pjw", bufs=1, side="right"))
                    projw_sb = pjw_pool.tile([128, DT, D], BF16, tag="projw")
                    nc.sync.dma_start(out=projw_sb, in_=wproj_t)
        if debug_out:
            nc.sync.dma_start(out=kto.ap(), in_=kt_all)
            nc.sync.dma_start(out=qto.ap(), in_=qt_all)
            nc.sync.dma_start(out=vpo.ap(), in_=VP)
            nc.sync.dma_start(out=aoto.ap(), in_=aoT)
        es_qkps.close()
        es_kqv.close()  # kt/qt/VP dead
        w1b_pool = es_w1b.enter_context(
            tc.tile_pool(name="w1bp", bufs=1, side="right"))
        w1b_sb = w1b_pool.tile([128, DT, HID], FP8, tag="w1bsb")
        nc.sync.dma_start(out=w1b_sb, in_=w1b_t)

        # ---- phase D+E interleaved: proj + residual -> x2, LN2 -> z2T ----
        es_x2 = ExitStack()
        x2p = es_x2.enter_context(tc.tile_pool(name="x2p", bufs=1))
        x2_all = x2p.tile([128, T_OWN // 128, D], FP32, tag="x2")
        es_z2 = ExitStack()
        z2p = es_z2.enter_context(tc.tile_pool(name="z2p", bufs=1, side="right"))
        z2T = z2p.tile([128, DT, T_OWN], FP8, tag="z2T")
        z2E = z2p.tile([128, DT, T_OWN], FP8, tag="z2E")
        with (
            tc.tile_pool(name="psD", bufs=2, space="PSUM") as ppsum,
            tc.tile_pool(name="psE", bufs=2, space="PSUM") as trpsE,
        ):
            for tt in range(T_OWN // 128):
                xo = xpool.tile([128, D], FP32, tag="xa")
                nc.sync.dma_start(out=xo, in_=x[tt * 128:(tt + 1) * 128, :])
                for oc in range(2):
                    ps = ppsum.tile([128, 512], FP32, tag="pps")
                    for d in range(DT):
                        nc.tensor.matmul(
                            ps, aoT[:, d, tt * 128:(tt + 1) * 128],
                            projw_sb[:, d, oc * 512:(oc + 1) * 512],
                            start=(d == 0), stop=(not with_bias and d == DT - 1),
                        )
                    if with_bias:
                        nc.tensor.matmul(
                            ps, ones_bf, bproj_sb[:, oc * 512:(oc + 1) * 512],
                            start=False, stop=True,
                        )
                    nc.vector.scalar_tensor_tensor(
                        out=x2_all[:, tt, oc * 512:(oc + 1) * 512],
                        in0=ps, scalar=0.0, in1=xo[:, oc * 512:(oc + 1) * 512],
                        op0=ALU.bypass, op1=ALU.add,
                    )
                ln_tile(x2_all[:, tt, :], trpsE, z2T, tt, res_dst=z2E)
            if debug_out:
                nc.sync.dma_start(out=x2o.ap(), in_=x2_all)

        # ---- phase F: MLP ----
        # fc1: 12-member DoubleRow groups (z8*w1a + z8*w1b + zE*w1a); gelu
        # un-scales by 1/32. fc2: w2 streamed per output-column half, 32-member
        # sequential groups (h8*w2a + h8*w2b), output un-scaled by 1/64 in the
        # residual-add. Groups are strictly sequential (hardware corrupts
        # interleaved accumulation groups).
        with (
            tc.tile_pool(name="yp", bufs=3) as ypool,
            tc.tile_pool(name="hp", bufs=1) as hpool,
            tc.tile_pool(name="psF", bufs=3, space="PSUM") as fpsum,
        ):
            hT = hpool.tile([128, JT, T_OWN], FP8, tag="hT")
            for tc2 in range(2):
                for jt in range(JT):
                    ps = fpsum.tile([128, 512], FP32, tag="fps")
                    n = 0
                    for wsb, zt in ((w1a_sb, z2T), (w1b_sb, z2T),
                                    (w1a_sb, z2E)):
                        for dp in range(DP):
                            nc.tensor.matmul(
                                ps, wsb[:, 2 * dp:2 * dp + 2,
                                        jt * 128:(jt + 1) * 128],
                                zt[:, 2 * dp:2 * dp + 2,
                                   tc2 * 512:(tc2 + 1) * 512],
                                start=(n == 0), stop=(n == 11),
                                perf_mode=PM.DoubleRow)
                            n += 1
                    nc.scalar.activation(
                        out=hT[:, jt, tc2 * 512:(tc2 + 1) * 512], in_=ps,
                        func=AF.Gelu, bias=b1_sb[:, jt:jt + 1],
                        scale=1.0 / 32.0,
                    )
            # fc1 inputs dead; free before streaming w2 (LIFO right stack)
            es_z2.close()
            es_w1b.close()
            es_pjw.close()
            es_w1a.close()
            with (
                tc.tile_pool(name="w2s", bufs=2, side="right") as w2sp,
                tc.tile_pool(name="psF2", bufs=3, space="PSUM") as fpsum2,
            ):
                for oc in range(2):
                    w2oa = w2sp.tile([128, JT, 512], FP8, tag="w2oa")
                    nc.sync.dma_start(out=w2oa,
                                      in_=w2a_t[:, :, oc * 512:(oc + 1) * 512])
                    w2ob = w2sp.tile([128, JT, 512], FP8, tag="w2ob")
                    nc.sync.dma_start(out=w2ob,
                                      in_=w2b_t[:, :, oc * 512:(oc + 1) * 512])
                    for tglob in range(8):
                        ps2 = fpsum2.tile([128, 512], FP32, tag="fps2")
                        n = 0
                        for wsb in (w2oa, w2ob):
                            for jp in range(JP):
                                nc.tensor.matmul(
                                    ps2,
                                    hT[:, 2 * jp:2 * jp + 2,
                                       tglob * 128:(tglob + 1) * 128],
                                    wsb[:, 2 * jp:2 * jp + 2, :],
                                    start=(n == 0),
                                    stop=(not with_bias and n == 31),
                                    perf_mode=PM.DoubleRow)
                                n += 1
                        if with_bias:
                            nc.tensor.matmul(
                                ps2, ones_bf,
                                b2_sb[:, oc * 512:(oc + 1) * 512],
                                start=False, stop=True,
                            )
                        ys = ypool.tile([128, 512], FP32, tag="ys")
                        nc.vector.scalar_tensor_tensor(
                            out=ys, in0=ps2, scalar=1.0 / 64.0,
                            in1=x2_all[:, tglob, oc * 512:(oc + 1) * 512],
                            op0=ALU.mult, op1=ALU.add,
                        )
                        nc.sync.dma_start(
                            out=y[tglob * 128:(tglob + 1) * 128,
                                  oc * 512:(oc + 1) * 512],
                            in_=ys,
                        )
        es_x2.close()
        es_ao.close()

    nc.compile()
    return nc


def prep_host_inputs(inputs, with_bias, f8_attn=True):
    """Fold LN affine params into the adjacent matmul weights, quantize
    (fc weights scaled x32/x64 into fp8 base+residual pairs), and build the
    8 per-core input maps."""
    import ml_dtypes

    f32 = np.float32
    bf = ml_dtypes.bfloat16
    f8 = ml_dtypes.float8_e4m3
    x = np.asarray(inputs["x"], f32)
    qkv_w = np.asarray(inputs["qkv_w"], f32)
    qkv_b = np.asarray(inputs["qkv_b"], f32)
    proj_w = np.asarray(inputs["proj_w"], f32)
    fc1_w = np.asarray(inputs["fc1_w"], f32)
    fc1_b = np.asarray(inputs["fc1_b"], f32)
    fc2_w = np.asarray(inputs["fc2_w"], f32)
    ln1_w = np.asarray(inputs["ln1_w"], f32)
    ln1_b = np.asarray(inputs["ln1_b"], f32)
    ln2_w = np.asarray(inputs["ln2_w"], f32)
    ln2_b = np.asarray(inputs["ln2_b"], f32)

    w8 = f8 if f8_attn else bf
    wqkv = (ln1_w[:, None] * qkv_w).astype(w8)
    bqkv = (qkv_b + ln1_b @ qkv_w).astype(f32)
    w1s = (ln2_w[:, None] * fc1_w) * 32.0
    w1a = w1s.astype(f8)
    w1b = (w1s - w1a.astype(f32)).astype(f8)
    b1 = (fc1_b + ln2_b @ fc1_w).astype(f32)
    w2s = fc2_w * 64.0
    w2a = w2s.astype(f8)
    w2b = (w2s - w2a.astype(f32)).astype(f8)

    shared = {
        "wqkv": wqkv, "bqkv": bqkv,
        "wproj": proj_w.astype(bf),
        "w1a": w1a, "w1b": w1b, "b1": b1,
        "w2a": w2a, "w2b": w2b,
    }
    if with_bias:
        shared["bproj"] = np.asarray(inputs["proj_b"], f32).astype(bf)
        shared["b2"] = (np.asarray(inputs["fc2_b"], f32) * 64.0).astype(bf)
    in_maps = []
    for c in range(N_CORES):
        b, half = c // 2, c % 2
        own = x[b, half * 1024:(half + 1) * 1024]
        other = x[b, (1 - half) * 1024:(2 - half) * 1024]
        xc = np.concatenate([own, other], axis=0)
        in_maps.append({"x": np.ascontiguousarray(xc), **shared})
    return in_maps


# ---------------------------------------------------------------------------
# Cached PJRT runner (jit once per build variant, reuse across kernel() calls)
# ---------------------------------------------------------------------------
_CACHE = {}


def _get_runner(with_bias):
    key = ("runner", with_bias)
    if key in _CACHE:
        return _CACHE[key]

    from jax.experimental.shard_map import shard_map
    from jax.sharding import Mesh, PartitionSpec
    from concourse.bass2jax import (
        _bass_exec_p, install_neuronx_cc_hook, partition_id_tensor,
    )

    nc = build_nc(with_bias=with_bias)
    install_neuronx_cc_hook()

    partition_name = nc.partition_id_tensor.name if nc.partition_id_tensor else None
    in_names, out_names, out_avals, zero_shapes = [], [], [], []
    for alloc in nc.m.functions[0].allocations:
        if not isinstance(alloc, mybir.MemoryLocationSet):
            continue
        name = alloc.memorylocations[0].name
        if alloc.kind == "ExternalInput":
            if name != partition_name:
                in_names.append(name)
        elif alloc.kind == "ExternalOutput":
            shape = tuple(alloc.tensor_shape)
            dtype = mybir.dt.np(alloc.dtype)
            out_names.append(name)
            out_avals.append(jax.core.ShapedArray(shape, dtype))
            zero_shapes.append((shape, dtype))
    n_params = len(in_names)
    n_outs = len(out_names)
    all_in = list(in_names) + list(out_names)
    if partition_name is not None:
        all_in.append(partition_name)
    donate = tuple(range(n_params, n_params + n_outs))

    def _body(*args):
        operands = list(args)
        if partition_name is not None:
            operands.append(partition_id_tensor())
        outs = _bass_exec_p.bind(
            *operands,
            out_avals=tuple(out_avals),
            in_names=tuple(all_in),
            out_names=tuple(out_names),
            lowering_input_output_aliases=(),
            sim_require_finite=True,
            sim_require_nnan=True,
            nc=nc,
        )
        return tuple(outs)

    devices = jax.devices()[:N_CORES]
    mesh = Mesh(np.asarray(devices), ("core",))
    sharded = jax.jit(
        shard_map(
            _body, mesh=mesh,
            in_specs=(PartitionSpec("core"),) * (n_params + n_outs),
            out_specs=(PartitionSpec("core"),) * n_outs,
            check_rep=False,
        ),
        donate_argnums=donate, keep_unused=True,
    )

    def run(in_maps):
        concat_in = [
            np.concatenate([np.asarray(m[name]) for m in in_maps], axis=0)
            for name in in_names
        ]
        concat_zeros = [
            np.zeros((N_CORES * s[0], *s[1:]), dt) for (s, dt) in zero_shapes
        ]
        out_arrs = sharded(*concat_in, *concat_zeros)
        per_core = []
        for c in range(N_CORES):
            per_core.append({
                name: np.asarray(out_arrs[i]).reshape(
                    N_CORES, *out_avals[i].shape)[c]
                for i, name in enumerate(out_names)
            })
        return per_core

    _CACHE[key] = run
    return run


def kernel(**inputs) -> np.ndarray:
    with_bias = bool(
        np.any(np.asarray(inputs["proj_b"])) or np.any(np.asarray(inputs["fc2_b"]))
    )
    run = _get_runner(with_bias)
    in_maps = prep_host_inputs(inputs, with_bias)
    results = run(in_maps)
    out = np.zeros((4, 2048, 1024), np.float32)
    for c in range(N_CORES):
        b, half = c // 2, c % 2
        out[b, half * 1024:(half + 1) * 1024] = results[c]["y"]
    return out
